# revision 8
# baseline (speedup 1.0000x reference)
"""Masked cross-attention (flamingo-style) Trainium2 Bass kernel.

Full inputs -> full output. Sharding: 8 cores = batch(4) x text-half(2).
Each core computes LayerNorm -> q/kv projections -> masked softmax
attention over media -> output projection for its 1024 text rows.

All heavy matmuls run in float32r (tf32-like, full PE rate at N>=256).
The location/existence mask is folded on host into a single {1e30, 1e-30}
bf16 "allowance" tensor: exp' = min(exp(sim) + 1e-30, allow), which also
reproduces the reference's uniform-softmax behavior for fully-masked rows.
"""
import sys

sys.path.insert(0, "/opt/trn_rl_repo")

import numpy as np
import ml_dtypes

import concourse.bacc as bacc
import concourse.mybir as mybir
import concourse.tile as tile
from concourse.bass_utils import run_bass_kernel_spmd
from concourse.masks import make_identity

F32 = mybir.dt.float32
F32R = mybir.dt.float32r
BF16 = mybir.dt.bfloat16
AF = mybir.ActivationFunctionType
OP = mybir.AluOpType

B, N, DIM = 4, 2048, 1024
HEADS, DH, INNER = 8, 64, 512
T, M = 8, 64
TM = T * M                       # 512 media positions
R = N // 2                       # 1024 rows per core
EPS = 1e-5
TINY = 1e-30

_CACHED_NC = None


def build():
    nc = bacc.Bacc("TRN2", target_bir_lowering=False, debug=False)

    xh = nc.dram_tensor("xh", [R, DIM], F32, kind="ExternalInput")
    med = nc.dram_tensor("med", [TM, DIM], F32, kind="ExternalInput")
    wq = nc.dram_tensor("wq", [DIM, INNER], F32R, kind="ExternalInput")
    bq = nc.dram_tensor("bq", [4, 128], F32, kind="ExternalInput")
    wkv = nc.dram_tensor("wkv", [DIM, 2 * INNER], F32R, kind="ExternalInput")
    wo = nc.dram_tensor("wo", [INNER, DIM], F32R, kind="ExternalInput")
    eqb = nc.dram_tensor("eqb", [TM, R], BF16, kind="ExternalInput")
    y = nc.dram_tensor("y", [R, DIM], F32, kind="ExternalOutput")

    with tile.TileContext(nc) as tc:
        _program(nc, tc, xh, med, wq, bq, wkv, wo, eqb, y)
    nc.compile()
    return nc


def _program(nc, tc, xh, med, wq, bq, wkv, wo, eqb, y):
    from contextlib import ExitStack

    ctx = ExitStack()
    with ctx:
        singles = ctx.enter_context(tc.tile_pool(name="singles", bufs=1))
        stage = ctx.enter_context(tc.tile_pool(name="stage", bufs=2))
        small = ctx.enter_context(tc.tile_pool(name="small", bufs=6))
        expool = ctx.enter_context(tc.tile_pool(name="expool", bufs=6))
        oapool = ctx.enter_context(tc.tile_pool(name="oapool", bufs=5))
        rpool = ctx.enter_context(tc.tile_pool(name="rpool", bufs=8))
        ypool = ctx.enter_context(tc.tile_pool(name="ypool", bufs=2))
        ppb = ctx.enter_context(tc.tile_pool(name="ppb", bufs=4, space="PSUM"))
        ppa = ctx.enter_context(tc.tile_pool(name="ppa", bufs=4, space="PSUM"))

        # ---- resident tensors ----
        wq_sb = singles.tile([128, 8, INNER], F32R)
        nc.sync.dma_start(out=wq_sb, in_=wq.rearrange("(dc p) c -> p dc c", p=128))
        wkv_sb = singles.tile([128, 8, 2 * INNER], F32R)
        nc.sync.dma_start(out=wkv_sb, in_=wkv.rearrange("(dc p) c -> p dc c", p=128))
        wo_sb = singles.tile([128, 4, DIM], F32R)
        nc.sync.dma_start(out=wo_sb, in_=wo.rearrange("(cc p) d -> p cc d", p=128))
        bq_sb = singles.tile([128, 4], F32)
        nc.sync.dma_start(out=bq_sb, in_=bq.rearrange("cc p -> p cc"))
        eq_sb = singles.tile([128, 4, R], BF16)
        nc.sync.dma_start(out=eq_sb, in_=eqb.rearrange("(jc p) i -> p jc i", p=128))

        zT = singles.tile([128, 8, R], F32R)       # [d-in-chunk, dc, i]
        mediaT = singles.tile([128, 8, TM], F32R)  # [d-in-chunk, dc, j]
        qT = singles.tile([128, 4, R], F32R)       # [c-in-chunk, cc, i]
        kT = singles.tile([128, 4, TM], F32R)      # [c-in-chunk, cc, j]
        va = singles.tile([128, 4, HEADS, DH + 4], F32R)  # [j-in-chunk, jc, h, dh|1,1,1,1]
        oT = singles.tile([128, 4, R], F32R)       # [c-in-chunk, cc, i]

        id32 = singles.tile([128, 128], F32)
        make_identity(nc, id32)
        idr = singles.tile([128, 128], F32R)
        nc.vector.tensor_copy(out=idr, in_=id32)
        ones = singles.tile([128, 1], F32)
        nc.vector.memset(ones, 1.0)
        epsb = singles.tile([128, 1], F32)
        nc.vector.memset(epsb, EPS)

        # ---- phase A: LayerNorm + transpose into zT ----
        for r in range(8):
            xt = stage.tile([128, DIM], F32, tag="stage")
            nc.sync.dma_start(out=xt, in_=xh[r * 128:(r + 1) * 128, :])
            st = small.tile([128, 2, 6], F32, tag="bns")
            x2 = xt.rearrange("p (g f) -> p g f", g=2)
            for g in range(2):
                nc.vector.bn_stats(out=st[:, g, :], in_=x2[:, g, :])
            mv = small.tile([128, 2], F32, tag="mv")
            nc.vector.bn_aggr(out=mv, in_=st)
            srt = small.tile([128, 1], F32, tag="srt")
            nc.scalar.activation(out=srt, in_=mv[:, 1:2], func=AF.Sqrt,
                                 bias=epsb[:, :])
            rstd = small.tile([128, 1], F32, tag="rstd")
            nc.vector.reciprocal(out=rstd, in_=srt)
            # z = (x - mean) * rstd, in place
            nc.vector.tensor_scalar(out=xt, in0=xt, scalar1=mv[:, 0:1],
                                    scalar2=rstd, op0=OP.subtract, op1=OP.mult)
            for g in range(2):
                pt = ppb.tile([128, 512], F32, tag="big")
                for k in range(4):
                    dc = g * 4 + k
                    nc.tensor.transpose(pt[:, k * 128:(k + 1) * 128],
                                        xt[:, dc * 128:(dc + 1) * 128], id32)
                dst = zT[:, g * 4:(g + 1) * 4, r * 128:(r + 1) * 128]
                nc.scalar.copy(out=dst, in_=pt.rearrange("p (k q) -> p k q", k=4))

        # ---- phase B: transpose media into mediaT ----
        for js in range(4):
            mt_ = stage.tile([128, DIM], F32, tag="stage")
            nc.sync.dma_start(out=mt_, in_=med[js * 128:(js + 1) * 128, :])
            for g in range(2):
                pt = ppb.tile([128, 512], F32, tag="big")
                for k in range(4):
                    dc = g * 4 + k
                    nc.tensor.transpose(pt[:, k * 128:(k + 1) * 128],
                                        mt_[:, dc * 128:(dc + 1) * 128], id32)
                dst = mediaT[:, g * 4:(g + 1) * 4, js * 128:(js + 1) * 128]
                nc.scalar.copy(out=dst, in_=pt.rearrange("p (k q) -> p k q", k=4))

        # ---- phase C: q projection (qT = wq^T @ zT + bq) ----
        for cc in range(4):
            for it in range(2):
                pq = ppb.tile([128, 512], F32, tag="big")
                for dc in range(8):
                    nc.tensor.matmul(pq, wq_sb[:, dc, cc * 128:(cc + 1) * 128],
                                     zT[:, dc, it * 512:(it + 1) * 512],
                                     start=(dc == 0), stop=(dc == 7))
                nc.vector.tensor_scalar(out=qT[:, cc, it * 512:(it + 1) * 512],
                                        in0=pq, scalar1=bq_sb[:, cc:cc + 1],
                                        scalar2=None, op0=OP.add)

        # ---- phase D: k/v projections ----
        for cc in range(4):
            pk = ppb.tile([128, 512], F32, tag="big")
            for dc in range(8):
                nc.tensor.matmul(pk, wkv_sb[:, dc, cc * 128:(cc + 1) * 128],
                                 mediaT[:, dc, :],
                                 start=(dc == 0), stop=(dc == 7))
            nc.scalar.copy(out=kT[:, cc, :], in_=pk)
        for js in range(4):
            pv = ppb.tile([128, 512], F32, tag="big")
            for dc in range(8):
                nc.tensor.matmul(pv, mediaT[:, dc, js * 128:(js + 1) * 128],
                                 wkv_sb[:, dc, INNER:2 * INNER],
                                 start=(dc == 0), stop=(dc == 7))
            nc.scalar.copy(out=va[:, js, :, 0:DH],
                           in_=pv.rearrange("p (h d) -> p h d", h=HEADS))
            nc.vector.tensor_copy(out=va[:, js, :, DH:DH + 4],
                                  in_=ones.to_broadcast([128, HEADS, 4]))

        # ---- phase E: attention ----
        for it in range(2):
            oal = [oapool.tile([128, INNER], F32R, tag="oa", name=f"oa{it}_{s}")
                   for s in range(4)]
            for h in range(HEADS):
                cc, hf = h // 2, (h % 2) * 64
                exl = []
                for jc in range(4):
                    psim = ppb.tile([128, 512], F32, tag="big")
                    nc.tensor.matmul(psim,
                                     kT[hf:hf + 64, cc, jc * 128:(jc + 1) * 128],
                                     qT[hf:hf + 64, cc, it * 512:(it + 1) * 512],
                                     start=True, stop=True)
                    ex = expool.tile([128, 512], F32R, tag="ex")
                    nc.scalar.activation(out=ex, in_=psim, func=AF.Exp)
                    nc.vector.scalar_tensor_tensor(
                        out=ex, in0=ex, scalar=TINY,
                        in1=eq_sb[:, jc, it * 512:(it + 1) * 512],
                        op0=OP.add, op1=OP.min)
                    exl.append(ex)
                for s in range(4):
                    pav = ppa.tile([128, DH + 4], F32, tag="av")
                    for jc in range(4):
                        nc.tensor.matmul(pav,
                                         exl[jc][:, s * 128:(s + 1) * 128],
                                         va[:, jc, h, :],
                                         start=(jc == 0), stop=(jc == 3))
                    rec = rpool.tile([128, 1], F32, tag="rec")
                    nc.vector.reciprocal(out=rec, in_=pav[:, DH:DH + 1])
                    nc.vector.tensor_scalar(out=oal[s][:, h * 64:(h + 1) * 64],
                                            in0=pav[:, 0:DH], scalar1=rec,
                                            scalar2=None, op0=OP.mult)
            # transpose finished islices into oT
            for s in range(4):
                isl = it * 4 + s
                pt = ppb.tile([128, 512], F32R, tag="big")
                for k in range(4):
                    nc.tensor.transpose(pt[:, k * 128:(k + 1) * 128],
                                        oal[s][:, k * 128:(k + 1) * 128],
                                        idr)
                dst = oT[:, 0:4, isl * 128:(isl + 1) * 128]
                nc.scalar.copy(out=dst, in_=pt.rearrange("p (k q) -> p k q", k=4))

        # ---- phase F: output projection ----
        for s2 in range(8):
            for dhh in range(2):
                py = ppb.tile([128, 512], F32, tag="big")
                for cc in range(4):
                    nc.tensor.matmul(py, oT[:, cc, s2 * 128:(s2 + 1) * 128],
                                     wo_sb[:, cc, dhh * 512:(dhh + 1) * 512],
                                     start=(cc == 0), stop=(cc == 3))
                yt = ypool.tile([128, 512], F32, tag="y")
                nc.scalar.copy(out=yt, in_=py)
                nc.sync.dma_start(
                    out=y[s2 * 128:(s2 + 1) * 128, dhh * 512:(dhh + 1) * 512],
                    in_=yt)


def _prep(x, media, media_locations, aug_exist_idx, ln_gamma, ln_beta,
          Wq, Wkv, Wo):
    scale = DH ** -0.5
    wq_eff = np.ascontiguousarray((ln_gamma[:, None] * Wq) * scale,
                                  dtype=np.float32)
    bq_eff = np.ascontiguousarray(((ln_beta @ Wq) * scale).reshape(4, 128),
                                  dtype=np.float32)
    wkv_c = np.ascontiguousarray(Wkv, dtype=np.float32)
    wo_c = np.ascontiguousarray(Wo, dtype=np.float32)

    tt = np.cumsum(media_locations.astype(np.int64), axis=-1)  # [B, N]
    mt = np.repeat(np.arange(T, dtype=np.int64) + 1, M)        # [TM]
    mtp = np.where(aug_exist_idx.astype(np.int64) == 1,
                   np.arange(T, dtype=np.int64) + 1, -1)       # [B, T]
    mtp = np.repeat(mtp, M, axis=-1)                           # [B, TM]
    allow = np.where(mtp[:, :, None] == tt[:, None, :], 1e30, TINY)
    allow = allow.astype(ml_dtypes.bfloat16)                   # [B, TM, N]

    in_maps = []
    for b in range(B):
        med_b = np.ascontiguousarray(media[b].reshape(TM, DIM),
                                     dtype=np.float32)
        for half in range(2):
            sl = slice(half * R, (half + 1) * R)
            in_maps.append({
                "xh": np.ascontiguousarray(x[b, sl, :], dtype=np.float32),
                "med": med_b,
                "wq": wq_eff,
                "bq": bq_eff,
                "wkv": wkv_c,
                "wo": wo_c,
                "eqb": np.ascontiguousarray(allow[b, :, sl]),
            })
    return in_maps


def kernel(x, media, media_locations, aug_exist_idx, ln_gamma, ln_beta,
           Wq, Wkv, Wo, _trace=False):
    global _CACHED_NC
    x = np.asarray(x)
    media = np.asarray(media)
    media_locations = np.asarray(media_locations)
    aug_exist_idx = np.asarray(aug_exist_idx)
    ln_gamma = np.asarray(ln_gamma)
    ln_beta = np.asarray(ln_beta)
    Wq = np.asarray(Wq)
    Wkv = np.asarray(Wkv)
    Wo = np.asarray(Wo)

    if _CACHED_NC is None:
        _CACHED_NC = build()
    nc = _CACHED_NC

    in_maps = _prep(x, media, media_locations, aug_exist_idx, ln_gamma,
                    ln_beta, Wq, Wkv, Wo)
    kw = {}
    if _trace:
        kw = dict(trace=True)
    res = run_bass_kernel_spmd(nc, in_maps, core_ids=list(range(8)), **kw)
    out = np.empty((B, N, DIM), dtype=np.float32)
    ci = 0
    for b in range(B):
        for half in range(2):
            out[b, half * R:(half + 1) * R, :] = res.results[ci]["y"]
            ci += 1
    if _trace:
        kernel._last_results = res
    return out
